# revision 7
# baseline (speedup 1.0000x reference)
"""CPC unsupervised criterion (losses, acc) on 8 Trainium2 NeuronCores.

Strategy (data-parallel over the nGtSequence axis g, one sequence per core):
  - Host stages per-core inputs: cT = cFeature[g,:128].T / 256 (a on partitions,
    pre-scaled so every downstream dot product carries the reference's /dimE),
    wT = W with the contraction axis a on partitions, posAll = the 12 shifted
    gtPredictions windows, the negative pool cast to bf16, and the gather
    indices converted to the SWDGE int16 wrapped layout.
  - On device: locC is built twice on the PE (once as [w,e] for the positive
    scores, once transposed to [e,(w,k)] bf16 for the negative matmuls).
    Negative rows arrive via dma_gather(transpose=True) already in the
    [e(part), n(free)] layout the PE contracts over — no on-chip transposes
    and half the HBM traffic of an f32 gather.
  - Per window w, a 3-matmul PSUM group computes diff[n, k] =
    scoreNeg - scorePos (third matmul is a rank-1 update with lhsT = -ones and
    rhs = the scorePos row). Sums over n (for the softmax denominator and the
    argmax-beat count) are ones-vector matmuls, avoiding partition reductions.
  - loss_k = mean_w log1p(sum_n exp(diff)), acc_k = mean_w [count(diff>0)==0];
    the host averages the 8 per-core partials.
"""

import numpy as np
import ml_dtypes

G, S, D, K, NEG, POOL = 8, 140, 256, 12, 128, 20000
W = S - K              # 128 window positions
NCORES = 8
NIDX = NEG * W         # 16384 gathered rows per core
CHUNKS = 8             # gather chunks (16 w each -> 2048 rows)
IDX_PER_CHUNK = NIDX // CHUNKS
WTILES = 4             # score tiles (32 w each)
WPT = W // WTILES      # 32 windows per score tile
FT = WPT * K           # 384 score columns per tile

_prog = None


def build_program():
    from contextlib import ExitStack
    import concourse.bacc as bacc
    import concourse.tile as tile
    import concourse.mybir as mybir
    from concourse.masks import make_identity

    f32 = mybir.dt.float32
    bf16 = mybir.dt.bfloat16
    i16 = mybir.dt.int16
    AX = mybir.AxisListType
    OP = mybir.AluOpType
    AF = mybir.ActivationFunctionType

    nc = bacc.Bacc("TRN2", target_bir_lowering=False, debug=False,
                   num_devices=NCORES)
    cT = nc.dram_tensor("cT", [128, 2, W], f32, kind="ExternalInput").ap()
    wT = nc.dram_tensor("wT", [128, K, 2, D], f32, kind="ExternalInput").ap()
    posA = nc.dram_tensor("posA", [W, K, D], f32, kind="ExternalInput").ap()
    poolB = nc.dram_tensor("poolB", [POOL, D], bf16, kind="ExternalInput").ap()
    idxT = nc.dram_tensor("idxT", [128, NIDX // 16], i16,
                          kind="ExternalInput").ap()
    outD = nc.dram_tensor("outD", [1, 2 * K], f32, kind="ExternalOutput").ap()

    with ExitStack() as ctx:
        tc = ctx.enter_context(tile.TileContext(nc))
        const = ctx.enter_context(tc.tile_pool(name="const", bufs=1))
        work = ctx.enter_context(tc.tile_pool(name="work", bufs=1))
        scr = ctx.enter_context(tc.tile_pool(name="scr", bufs=2))
        negp = ctx.enter_context(tc.tile_pool(name="negp", bufs=4))
        expp = ctx.enter_context(tc.tile_pool(name="expp", bufs=2))
        psA = ctx.enter_context(tc.tile_pool(name="psA", bufs=2, space="PSUM"))
        psB = ctx.enter_context(tc.tile_pool(name="psB", bufs=2, space="PSUM"))
        psS = ctx.enter_context(tc.tile_pool(name="psS", bufs=2, space="PSUM"))
        psR = ctx.enter_context(tc.tile_pool(name="psR", bufs=2, space="PSUM"))

        cT_s = const.tile([128, 2, W], f32)
        nc.sync.dma_start(cT_s[:], cT[:])
        wT_s = const.tile([128, K, 2, D], f32)
        nc.sync.dma_start(wT_s[:], wT[:])
        pos_s = const.tile([W, K, D], f32)
        nc.sync.dma_start(pos_s[:], posA[:])
        idx_s = const.tile([128, NIDX // 16], i16)
        nc.sync.dma_start(idx_s[:], idxT[:])
        ident = const.tile([128, 128], f32)
        make_identity(nc, ident[:])
        ones128 = const.tile([128, 1], f32)
        nc.vector.memset(ones128[:], 1.0)
        negOne = const.tile([1, 128], f32)
        nc.vector.memset(negOne[:], -1.0)

        locA_s = work.tile([W, K, D], f32)
        locBT_s = work.tile([128, 2, W, K], bf16)
        sPos = work.tile([W, K], f32)
        SC_all = work.tile([1, 2, WTILES, FT], f32)
        res = work.tile([1, 2 * K], f32)
        out_s = work.tile([1, 2 * K], f32)

        # Negative-row gathers, issued up front so DMA overlaps phase 1.
        negs = []
        for c in range(CHUNKS):
            ng = negp.tile([128, 2, IDX_PER_CHUNK], bf16)
            nc.gpsimd.dma_gather(
                ng[:], poolB[:], idx_s[:, 128 * c:128 * (c + 1)],
                IDX_PER_CHUNK, IDX_PER_CHUNK, D, transpose=True)
            negs.append(ng)

        # Phase 1a: locC[w, e] per k (inputs pre-scaled by 1/D on host).
        for k in range(K):
            pa = psA.tile([W, D], f32)
            for ac in range(2):
                nc.tensor.matmul(pa[:], lhsT=cT_s[:, ac, :],
                                 rhs=wT_s[:, k, ac, :],
                                 start=(ac == 0), stop=(ac == 1))
            nc.scalar.copy(locA_s[:, k, :], pa[:])

        # Phase 1b: locC_T[e, (w, k)] in bf16 via PE transpose.
        for k in range(K):
            for ec in range(2):
                pb = psB.tile([128, 128], f32)
                nc.tensor.transpose(pb[:], locA_s[:, k, 128 * ec:128 * (ec + 1)],
                                    ident[:])
                nc.vector.tensor_copy(locBT_s[:, ec, :, k], pb[:])

        # Phase 1c: sPos[w, k] = scorePos, then linearized to one partition so
        # per-window rows can be addressed at a legal start partition.
        for k in range(K):
            sc = scr.tile([W, D], f32)
            nc.vector.tensor_tensor_reduce(
                out=sc[:], in0=locA_s[:, k, :], in1=pos_s[:, k, :],
                scale=1.0, scalar=0.0, op0=OP.mult, op1=OP.add,
                accum_out=sPos[:, k:k + 1])
        sPosLin = work.tile([1, W * K], f32)
        nc.sync.dma_start(sPosLin[:], sPos[:])

        # Phase 2: diff[n, k] = scoreNeg - scorePos per window, then
        # sum(exp(diff)) and count(diff > 0) over n via ones-matmuls.
        for t in range(WTILES):
            ps = psS.tile([128, WPT, K], f32)
            for wc in range(WPT):
                w = WPT * t + wc
                po = ps[:, wc, :]
                cidx, l16 = w // 16, w % 16
                for ec in range(2):
                    nc.tensor.matmul(
                        po, lhsT=negs[cidx][:, ec, 128 * l16:128 * (l16 + 1)],
                        rhs=locBT_s[:, ec, w, :],
                        start=(ec == 0), stop=False)
                nc.tensor.matmul(po, lhsT=negOne[:],
                                 rhs=sPosLin[:, K * w:K * (w + 1)],
                                 start=False, stop=True, tile_position=(0, 0))
            ex = expp.tile([128, FT], f32, tag="ex")
            nc.scalar.activation(ex[:], ps[:], AF.Exp)
            ind = expp.tile([128, FT], f32, tag="ind")
            nc.vector.tensor_scalar(out=ind[:], in0=ps[:], scalar1=0.0,
                                    scalar2=None, op0=OP.is_gt)
            pr = psR.tile([1, FT], f32)
            nc.tensor.matmul(pr[:], lhsT=ones128[:], rhs=ex[:],
                             start=True, stop=True)
            nc.scalar.copy(SC_all[:, 0, t, :], pr[:])
            pr = psR.tile([1, FT], f32)
            nc.tensor.matmul(pr[:], lhsT=ones128[:], rhs=ind[:],
                             start=True, stop=True)
            nc.vector.tensor_copy(SC_all[:, 1, t, :], pr[:])

        # Phase 3: fold over w; res = [log1p-sums | beat-free counts] / W.
        s1 = work.tile([1, W, K], f32)
        nc.vector.tensor_scalar_add(
            s1[:], SC_all[:, 0, :, :].rearrange("p t (w k) -> p (t w) k", k=K),
            1.0)
        lns = work.tile([1, W, K], f32)
        nc.scalar.activation(lns[:], s1[:], AF.Ln)
        nc.vector.tensor_reduce(
            out=res[:, 0:K], in_=lns[:].rearrange("p w k -> p k w"),
            op=OP.add, axis=AX.X)
        ind2 = work.tile([1, W, K], f32)
        nc.vector.tensor_scalar(
            out=ind2[:],
            in0=SC_all[:, 1, :, :].rearrange("p t (w k) -> p (t w) k", k=K),
            scalar1=0.0, scalar2=None, op0=OP.is_equal)
        nc.vector.tensor_reduce(
            out=res[:, K:2 * K], in_=ind2[:].rearrange("p w k -> p k w"),
            op=OP.add, axis=AX.X)
        nc.scalar.mul(out_s[:], res[:], 1.0 / W)
        nc.sync.dma_start(outD[:], out_s[:])

    nc.compile()
    return nc


def stage_inputs(inputs):
    cF = np.asarray(inputs["cFeature"], dtype=np.float32)
    gt = np.asarray(inputs["gtPredictions"], dtype=np.float32)
    pool = np.asarray(inputs["otherEncoded"], dtype=np.float32)
    idx = np.asarray(inputs["extIdx"])
    Wm = np.asarray(inputs["W"], dtype=np.float32)

    poolB = np.ascontiguousarray(pool.astype(ml_dtypes.bfloat16))
    wT = np.ascontiguousarray(
        Wm.transpose(2, 0, 1).reshape(2, 128, K, D).transpose(1, 2, 0, 3))

    in_maps = []
    for g in range(G):
        cT = (cF[g, :W].T / np.float32(D)).reshape(2, 128, W).transpose(1, 0, 2)
        posA = np.stack([gt[g, kk + 1:kk + 1 + W] for kk in range(K)], axis=1)
        flat = idx[g].astype(np.int16).T.reshape(-1)   # order i = w*128 + n
        idx16 = np.zeros((128, NIDX // 16), np.int16)
        idx16[:16] = flat.reshape(-1, 16).T
        in_maps.append({
            "cT": np.ascontiguousarray(cT),
            "wT": wT,
            "posA": np.ascontiguousarray(posA),
            "poolB": poolB,
            "idxT": idx16,
        })
    return in_maps


def kernel(**inputs):
    global _prog
    if _prog is None:
        _prog = build_program()
    from concourse.bass_utils import run_bass_kernel_spmd
    in_maps = stage_inputs(inputs)
    res = run_bass_kernel_spmd(_prog, in_maps, list(range(NCORES))).results
    outs = np.stack([res[i]["outD"][0] for i in range(NCORES)]).astype(np.float64)
    m = outs.mean(axis=0)
    return (np.ascontiguousarray(m[:K]).astype(np.float32),
            np.ascontiguousarray(m[K:]).astype(np.float32))
